# revision 10
# baseline (speedup 1.0000x reference)
"""Trainium2 Bass kernel for nn_DCGN_5239860101881.

Math background (verified against the reference numerically):
  - The DCGN's "adjacency" matrix is diagonal with diag == 1.0 in fp32
    (cos(v,v) path), so einsum('xyz,abc->xbc') makes every propagate output
      out[b] = S * (sum_batch(node_conv(x)) @ W) + bias      (S = 360 / 120)
    and the reference output consists of 64 bit-identical [40,10] blocks.
  - The only computation touching the big x tensor is x.sum(axis=0).

Distribution: shard the node axis (1080 = 8 * 135) across the 8 cores.
Each core streams its [64, 135, 512] slice from HBM (DMA-bound), reduces
over batch, then runs the tiny replicated chain:
  node_conv -> @prop1_W -> gelu(360*. + b1) -> node_conv2(64x folded into w)
  -> @prop2_W -> gelu(120*. + b2) -> classifier
producing 5 of the 40 distinct output rows. No collectives needed.
"""

import os
import numpy as np

B, N, F = 64, 1080, 512
H1, H2, NCLS = 784, 28, 10
P = 3
NCORES = 8
SLICE_N = N // NCORES            # 135 nodes per core
NW = SLICE_N // P                # 45 layer-1 windows per core
S2 = NW // P                     # 15 layer-2 windows per core
CR = S2 // P                     # 5 classifier rows per core
SLICE_ELEMS = SLICE_N * F        # 69120
FLAT_P, FLAT_F = 128, SLICE_ELEMS // 128   # [128, 540] flat accumulator
GB = 8                           # batches per DMA group
NGROUPS = B // GB

_CACHE = {}


def _build_bass():
    import concourse.mybir as mybir
    from concourse import bacc
    from concourse.tile import TileContext

    fp32 = mybir.dt.float32
    nc = bacc.Bacc("TRN2", target_bir_lowering=False, debug=False,
                   num_devices=NCORES)

    # Per-core inputs (xs differs per core; weights identical).
    xs = nc.dram_tensor("xs", [B, SLICE_ELEMS], fp32, kind="ExternalInput")
    ncw1rep = nc.dram_tensor("ncw1rep", [NW, P * F], fp32, kind="ExternalInput")
    p1wr = nc.dram_tensor("p1wr", [128, 4, H1], fp32, kind="ExternalInput")
    b1r = nc.dram_tensor("b1r", [112, 7], fp32, kind="ExternalInput")
    nc2wr = nc.dram_tensor("nc2wr", [112, 7, P], fp32, kind="ExternalInput")
    p2wr = nc.dram_tensor("p2wr", [112, 7, H2], fp32, kind="ExternalInput")
    b2c = nc.dram_tensor("b2c", [H2, 1], fp32, kind="ExternalInput")
    cw1 = nc.dram_tensor("cw1", [H2, P, 32], fp32, kind="ExternalInput")
    cb1c = nc.dram_tensor("cb1c", [32, 1], fp32, kind="ExternalInput")
    cw2 = nc.dram_tensor("cw2", [32, NCLS], fp32, kind="ExternalInput")
    cb2c = nc.dram_tensor("cb2c", [NCLS, 1], fp32, kind="ExternalInput")
    eye45 = nc.dram_tensor("eye45", [NW, NW], fp32, kind="ExternalInput")

    out = nc.dram_tensor("out", [NCLS, CR], fp32, kind="ExternalOutput")

    Gelu = mybir.ActivationFunctionType.Gelu
    Ident = mybir.ActivationFunctionType.Identity

    with TileContext(nc) as tc:
        with (
            tc.tile_pool(name="w", bufs=1) as wpool,
            tc.tile_pool(name="stream", bufs=3) as spool,
            tc.tile_pool(name="acc", bufs=1) as apool,
            tc.tile_pool(name="tail", bufs=1) as tpool,
            tc.tile_pool(name="psum", bufs=4, space="PSUM") as psum,
            tc.tile_pool(name="dram", bufs=1, space="DRAM") as dpool,
        ):
            # ---- weight preloads (overlap with phase A) ----
            ncw1_sb = wpool.tile([NW, P * F], fp32)
            nc.sync.dma_start(out=ncw1_sb, in_=ncw1rep.ap())
            p1w_sb = wpool.tile([128, 4, H1], fp32)
            nc.sync.dma_start(out=p1w_sb, in_=p1wr.ap())
            b1_sb = wpool.tile([112, 7], fp32)
            nc.sync.dma_start(out=b1_sb, in_=b1r.ap())
            nc2w_sb = wpool.tile([112, 7, P], fp32)
            nc.sync.dma_start(out=nc2w_sb, in_=nc2wr.ap())
            p2w_sb = wpool.tile([112, 7, H2], fp32)
            nc.sync.dma_start(out=p2w_sb, in_=p2wr.ap())
            b2_sb = wpool.tile([H2, 1], fp32)
            nc.sync.dma_start(out=b2_sb, in_=b2c.ap())
            cw1_sb = wpool.tile([H2, P, 32], fp32)
            nc.sync.dma_start(out=cw1_sb, in_=cw1.ap())
            cb1_sb = wpool.tile([32, 1], fp32)
            nc.sync.dma_start(out=cb1_sb, in_=cb1c.ap())
            cw2_sb = wpool.tile([32, NCLS], fp32)
            nc.sync.dma_start(out=cw2_sb, in_=cw2.ap())
            cb2_sb = wpool.tile([NCLS, 1], fp32)
            nc.sync.dma_start(out=cb2_sb, in_=cb2c.ap())
            eye_sb = wpool.tile([NW, NW], fp32)
            nc.sync.dma_start(out=eye_sb, in_=eye45.ap())

            # ---- phase A: stream x slice, reduce over batch ----
            acc = apool.tile([FLAT_P, FLAT_F], fp32)
            for g in range(NGROUPS):
                gt = spool.tile([FLAT_P, GB, FLAT_F], fp32, tag="grp")
                src = xs.ap()[g * GB:(g + 1) * GB, :].rearrange(
                    "b (p f) -> p b f", p=FLAT_P)
                nc.sync.dma_start(out=gt, in_=src)
                for b in range(GB):
                    if g == 0 and b == 0:
                        nc.vector.tensor_copy(out=acc, in_=gt[:, 0, :])
                    else:
                        nc.vector.tensor_add(out=acc, in0=acc, in1=gt[:, b, :])

            # ---- phase B: tiny tail ----
            # reshuffle flat [128,540] -> window layout [45, 3*512] via DRAM
            scratch = dpool.tile([SLICE_ELEMS], fp32)
            nc.sync.dma_start(
                out=scratch.rearrange("(p f) -> p f", p=FLAT_P), in_=acc)
            xwin = tpool.tile([NW, P * F], fp32)
            nc.sync.dma_start(
                out=xwin, in_=scratch.rearrange("(w k) -> w k", w=NW))

            # Y = xwin * nc1_w (replicated)  [45, 1536]
            ywin = tpool.tile([NW, P * F], fp32)
            nc.vector.tensor_mul(out=ywin, in0=xwin, in1=ncw1_sb)

            # HsumT [128, 4, 45] via accumulating PE transposes of Y blocks
            hsumT = tpool.tile([128, 4, NW], fp32)
            for fc in range(4):
                pt = psum.tile([128, NW], fp32, tag="ps")
                for p in range(P):
                    blk = ywin[:, p * F + fc * 128: p * F + (fc + 1) * 128]
                    nc.tensor.matmul(pt, blk, eye_sb, is_transpose=True,
                                     start=(p == 0), stop=(p == P - 1))
                nc.vector.tensor_copy(out=hsumT[:, fc, :], in_=pt)

            # M1T chunks [112, 45] = prop1_W[:, chunk].T @ Hsum
            # then h1cT = gelu(360*M1T + b1)  -> [112, 7, 45]
            h1cT = tpool.tile([112, 7, NW], fp32)
            for hc in range(7):
                pm = psum.tile([112, NW], fp32, tag="ps")
                for fc in range(4):
                    lhsT = p1w_sb[:, fc, hc * 112:(hc + 1) * 112]
                    nc.tensor.matmul(pm, lhsT, hsumT[:, fc, :],
                                     start=(fc == 0), stop=(fc == 3))
                nc.scalar.activation(out=h1cT[:, hc, :], in_=pm, func=Gelu,
                                     bias=b1_sb[:, hc:hc + 1], scale=360.0)

            # layer2 node conv: Hs2T [112, 7, 15]
            # (64x batch-sum folded into nc2w on host)
            tmp2 = tpool.tile([112, 7, NW], fp32)
            h1v = h1cT.rearrange("p c (s q) -> p c s q", q=P)
            w2v = nc2w_sb[:, :, None, :].to_broadcast((112, 7, S2, P))
            nc.vector.tensor_mul(
                out=tmp2.rearrange("p c (s q) -> p c s q", q=P),
                in0=h1v, in1=w2v)
            hs2T = tpool.tile([112, 7, S2], fp32)
            nc.vector.reduce_sum(
                out=hs2T, in_=tmp2.rearrange("p c (s q) -> p c s q", q=P),
                axis=mybir.AxisListType.X)

            # M2T [28, 15] = prop2_W.T @ Hsum2 ; out2T = gelu(120*M2T + b2)
            pm2 = psum.tile([H2, S2], fp32, tag="ps")
            for c in range(7):
                nc.tensor.matmul(pm2, p2w_sb[:, c, :], hs2T[:, c, :],
                                 start=(c == 0), stop=(c == 6))
            out2T = tpool.tile([H2, S2], fp32)
            nc.scalar.activation(out=out2T, in_=pm2, func=Gelu,
                                 bias=b2_sb[:, 0:1], scale=120.0)

            # classifier layer 1 as 3 accumulating K=28 matmuls:
            # feat[84] rows q*28+h2 handled by lhsT = cls_w1[28q:28q+28, :]
            o2v = out2T.rearrange("h (r q) -> h r q", q=P)
            pc1 = psum.tile([32, CR], fp32, tag="ps")
            for q in range(P):
                nc.tensor.matmul(pc1, cw1_sb[:, q, :], o2v[:, :, q],
                                 start=(q == 0), stop=(q == P - 1))
            c1T = tpool.tile([32, CR], fp32)
            nc.scalar.activation(out=c1T, in_=pc1, func=Gelu,
                                 bias=cb1_sb[:, 0:1], scale=1.0)
            pc2 = psum.tile([NCLS, CR], fp32, tag="ps")
            nc.tensor.matmul(pc2, cw2_sb, c1T, start=True, stop=True)
            outT = tpool.tile([NCLS, CR], fp32)
            nc.scalar.activation(out=outT, in_=pc2, func=Ident,
                                 bias=cb2_sb[:, 0:1], scale=1.0)
            nc.sync.dma_start(out=out.ap(), in_=outT)

    nc.compile()
    return nc


def _prep_in_maps(inputs):
    x = np.ascontiguousarray(np.asarray(inputs["x"], dtype=np.float32))
    nc1_w = np.asarray(inputs["nc1_w"], dtype=np.float32)
    prop1_W = np.asarray(inputs["prop1_W"], dtype=np.float32)
    prop1_b = np.asarray(inputs["prop1_b"], dtype=np.float32)
    nc2_w = np.asarray(inputs["nc2_w"], dtype=np.float32)
    prop2_W = np.asarray(inputs["prop2_W"], dtype=np.float32)
    prop2_b = np.asarray(inputs["prop2_b"], dtype=np.float32)
    cls_w1 = np.asarray(inputs["cls_w1"], dtype=np.float32)
    cls_b1 = np.asarray(inputs["cls_b1"], dtype=np.float32)
    cls_w2 = np.asarray(inputs["cls_w2"], dtype=np.float32)
    cls_b2 = np.asarray(inputs["cls_b2"], dtype=np.float32)

    common = {
        "ncw1rep": np.ascontiguousarray(
            np.tile(nc1_w.reshape(1, P * F), (NW, 1))),
        "p1wr": np.ascontiguousarray(
            prop1_W.reshape(4, 128, H1).swapaxes(0, 1)),
        "b1r": np.ascontiguousarray(prop1_b.reshape(7, 112).T),
        "nc2wr": np.ascontiguousarray(
            (64.0 * nc2_w).astype(np.float32).T.reshape(7, 112, P)
            .swapaxes(0, 1)),
        "p2wr": np.ascontiguousarray(prop2_W.reshape(7, 112, H2)
                                     .swapaxes(0, 1)),
        "b2c": np.ascontiguousarray(prop2_b.reshape(H2, 1)),
        "cw1": np.ascontiguousarray(cls_w1.reshape(P, H2, 32).swapaxes(0, 1)),
        "cb1c": np.ascontiguousarray(cls_b1.reshape(32, 1)),
        "cw2": np.ascontiguousarray(cls_w2),
        "cb2c": np.ascontiguousarray(cls_b2.reshape(NCLS, 1)),
        "eye45": np.eye(NW, dtype=np.float32),
    }
    in_maps = []
    for c in range(NCORES):
        xs = np.ascontiguousarray(
            x[:, c * SLICE_N:(c + 1) * SLICE_N, :].reshape(B, SLICE_ELEMS))
        in_maps.append({"xs": xs, **common})
    return in_maps


def run(inputs, trace=False):
    from concourse import bass_utils
    if "nc" not in _CACHE:
        _CACHE["nc"] = _build_bass()
    nc = _CACHE["nc"]
    in_maps = _prep_in_maps(inputs)
    res = bass_utils.run_bass_kernel_spmd(
        nc, in_maps, core_ids=list(range(NCORES)), trace=trace)
    outs = [np.asarray(res.results[c]["out"]) for c in range(NCORES)]
    block = np.concatenate([o.T for o in outs], axis=0)       # [40, 10]
    full = np.tile(block, (B, 1)).astype(np.float32)          # [2560, 10]
    return full, res


def kernel(**inputs) -> np.ndarray:
    out, _ = run(inputs, trace=False)
    return out


# revision 13
# speedup vs baseline: 1.0541x; 1.0541x over previous
"""Trainium2 Bass kernel for nn_DCGN_5239860101881.

Math background (verified against the reference numerically):
  - The DCGN's "adjacency" matrix is diagonal with diag == 1.0 in fp32
    (cos(v,v) path), so einsum('xyz,abc->xbc') makes every propagate output
      out[b] = S * (sum_batch(node_conv(x)) @ W) + bias      (S = 360 / 120)
    and the reference output consists of 64 bit-identical [40,10] blocks.
  - The only computation touching the big x tensor is x.sum(axis=0).

Distribution: shard the node axis (1080 = 8 * 135) across the 8 cores.
Each core streams its [64, 135, 512] slice from HBM (DMA-bound), reduces
over batch, then runs the tiny replicated chain:
  node_conv -> @prop1_W -> gelu(360*. + b1) -> node_conv2(64x folded into w)
  -> @prop2_W -> gelu(120*. + b2) -> classifier
producing 5 of the 40 distinct output rows. No collectives needed.

Implementation notes:
  - Stream tiles are node-major [128 nodes, 512 f] per batch; the batch
    reduction runs on the otherwise-idle TensorE as accumulating PE
    transposes (psum += tile_chunk^T), which also leaves X^T in the
    f-on-partitions layout the tail matmuls need and keeps the PE HAM-warm.
  - Feature chunk 3 and the 7 leftover nodes (128..134) are reduced on DVE;
    leftovers stream in early and their reshuffle/transposes hide under
    phase A entirely.
"""

import numpy as np

B, N, F = 64, 1080, 512
H1, H2, NCLS = 784, 28, 10
P = 3
NCORES = 8
SLICE_N = N // NCORES            # 135 nodes per core
NW = SLICE_N // P                # 45 layer-1 windows per core
S2 = NW // P                     # 15 layer-2 windows per core
CR = S2 // P                     # 5 classifier rows per core
SLICE_ELEMS = SLICE_N * F        # 69120
MAIN_ELEMS = 128 * F             # 65536 (nodes 0..127)
LEFT_ELEMS = SLICE_ELEMS - MAIN_ELEMS  # 3584 (nodes 128..134)
GB = 8                           # batches per DMA group
NGROUPS = B // GB

_CACHE = {}


def _build_bass():
    import concourse.mybir as mybir
    from concourse import bacc
    from concourse.tile import TileContext

    fp32 = mybir.dt.float32
    nc = bacc.Bacc("TRN2", target_bir_lowering=False, debug=False,
                   num_devices=NCORES)

    xs = nc.dram_tensor("xs", [B, SLICE_ELEMS], fp32, kind="ExternalInput")
    nc1wT = nc.dram_tensor("nc1wT", [128, 4, P], fp32, kind="ExternalInput")
    eye128 = nc.dram_tensor("eye128", [128, 128], fp32, kind="ExternalInput")
    p1wr = nc.dram_tensor("p1wr", [128, 4, H1], fp32, kind="ExternalInput")
    b1r = nc.dram_tensor("b1r", [112, 7], fp32, kind="ExternalInput")
    nc2wr = nc.dram_tensor("nc2wr", [112, 7, P], fp32, kind="ExternalInput")
    p2wr = nc.dram_tensor("p2wr", [112, 7, H2], fp32, kind="ExternalInput")
    b2c = nc.dram_tensor("b2c", [H2, 1], fp32, kind="ExternalInput")
    cw1 = nc.dram_tensor("cw1", [H2, P, 32], fp32, kind="ExternalInput")
    cb1c = nc.dram_tensor("cb1c", [32, 1], fp32, kind="ExternalInput")
    cw2 = nc.dram_tensor("cw2", [32, NCLS], fp32, kind="ExternalInput")
    cb2c = nc.dram_tensor("cb2c", [NCLS, 1], fp32, kind="ExternalInput")

    out = nc.dram_tensor("out", [NCLS, CR], fp32, kind="ExternalOutput")

    Gelu = mybir.ActivationFunctionType.Gelu
    Ident = mybir.ActivationFunctionType.Identity

    with TileContext(nc) as tc:
        with (
            tc.tile_pool(name="w", bufs=1) as wpool,
            tc.tile_pool(name="stream", bufs=3) as spool,
            tc.tile_pool(name="left", bufs=1) as lpool,
            tc.tile_pool(name="acc", bufs=1) as apool,
            tc.tile_pool(name="tail", bufs=1) as tpool,
            tc.tile_pool(name="psA", bufs=1, space="PSUM") as psA,
            tc.tile_pool(name="psB", bufs=2, space="PSUM") as psB,
            tc.tile_pool(name="dram", bufs=1, space="DRAM") as dpool,
        ):
            # identity needed by the very first transposes
            eye_sb = wpool.tile([128, 128], fp32)
            nc.sync.dma_start(out=eye_sb, in_=eye128.ap())

            # leftover node stream (nodes 128..134), all batches, early
            llt = lpool.tile([128, B, 28], fp32)
            for q in range(4):
                src = xs.ap()[q * 16:(q + 1) * 16, MAIN_ELEMS:].rearrange(
                    "b (p f) -> p b f", p=128)
                nc.sync.dma_start(out=llt[:, q * 16:(q + 1) * 16, :], in_=src)

            # main group DMAs issued up-front in program order
            acc3 = apool.tile([128, 128], fp32)
            px = psA.tile([128, 3, 128], fp32)   # one psum bank
            gts = []
            for g in range(NGROUPS):
                gtm = spool.tile([128, GB, F], fp32, tag="grp")
                src = xs.ap()[g * GB:(g + 1) * GB, 0:MAIN_ELEMS].rearrange(
                    "b (n f) -> n b f", n=128)
                nc.sync.dma_start(out=gtm, in_=src)
                gts.append(gtm)
                for b in range(GB):
                    bg = g * GB + b
                    for fc in range(3):
                        nc.tensor.matmul(
                            px[:, fc, :], gtm[:, b, fc * 128:(fc + 1) * 128],
                            eye_sb, is_transpose=True,
                            start=(bg == 0 and fc == 0),
                            stop=(bg == B - 1 and fc == 2))
                    if bg == 0:
                        nc.vector.tensor_copy(out=acc3, in_=gtm[:, 0, 384:512])
                    else:
                        nc.vector.tensor_add(out=acc3, in0=acc3,
                                             in1=gtm[:, b, 384:512])

            # ---- weights (scheduled around the stream) ----
            nc1wT_sb = wpool.tile([128, 4, P], fp32)
            nc.sync.dma_start(out=nc1wT_sb, in_=nc1wT.ap())
            p1w_sb = wpool.tile([128, 4, H1], fp32)
            nc.sync.dma_start(out=p1w_sb, in_=p1wr.ap())
            b1_sb = wpool.tile([112, 7], fp32)
            nc.sync.dma_start(out=b1_sb, in_=b1r.ap())
            nc2w_sb = wpool.tile([112, 7, P], fp32)
            nc.sync.dma_start(out=nc2w_sb, in_=nc2wr.ap())
            p2w_sb = wpool.tile([112, 7, H2], fp32)
            nc.sync.dma_start(out=p2w_sb, in_=p2wr.ap())
            b2_sb = wpool.tile([H2, 1], fp32)
            nc.sync.dma_start(out=b2_sb, in_=b2c.ap())
            cw1_sb = wpool.tile([H2, P, 32], fp32)
            nc.sync.dma_start(out=cw1_sb, in_=cw1.ap())
            cb1_sb = wpool.tile([32, 1], fp32)
            nc.sync.dma_start(out=cb1_sb, in_=cb1c.ap())
            cw2_sb = wpool.tile([32, NCLS], fp32)
            nc.sync.dma_start(out=cw2_sb, in_=cw2.ap())
            cb2_sb = wpool.tile([NCLS, 1], fp32)
            nc.sync.dma_start(out=cb2_sb, in_=cb2c.ap())

            # preload the gelu ACT table during phase A
            gdummy = tpool.tile([H2, 1], fp32)
            nc.scalar.activation(out=gdummy, in_=b2_sb, func=Gelu)

            # X^T assembled here: [f-part 128, fc 4, node 135]
            xT_sb = tpool.tile([128, 4, SLICE_N], fp32)

            # leftover reduction: 63 adds of [128, 28] + roundtrip + transposes
            accl = apool.tile([128, 28], fp32)
            for b in range(B):
                if b == 0:
                    nc.vector.tensor_copy(out=accl, in_=llt[:, 0, :])
                else:
                    nc.vector.tensor_add(out=accl, in0=accl, in1=llt[:, b, :])
            scratch = dpool.tile([LEFT_ELEMS], fp32)
            nc.sync.dma_start(
                out=scratch.rearrange("(p f) -> p f", p=128), in_=accl)
            lt7 = lpool.tile([7, F], fp32)
            nc.sync.dma_start(
                out=lt7, in_=scratch.rearrange("(n f) -> n f", n=7))
            for fc in range(4):
                plt = psB.tile([128, 7], fp32, tag="plt")
                nc.tensor.matmul(plt, lt7[:, fc * 128:(fc + 1) * 128],
                                 eye_sb[:7, :7], is_transpose=True,
                                 start=True, stop=True)
                nc.vector.tensor_copy(out=xT_sb[:, fc, 128:135], in_=plt)

            # ---- drains after the stream ----
            for fc in range(3):
                nc.vector.tensor_copy(out=xT_sb[:, fc, 0:128], in_=px[:, fc, :])
            p3 = psB.tile([128, 128], fp32, tag="p3")
            nc.tensor.matmul(p3, acc3, eye_sb, is_transpose=True,
                             start=True, stop=True)
            nc.vector.tensor_copy(out=xT_sb[:, 3, 0:128], in_=p3)

            # ---- node conv 1 (transposed layout) ----
            tmp1 = tpool.tile([128, 4, SLICE_N], fp32)
            xv = xT_sb.rearrange("p c (s q) -> p c s q", q=P)
            wv = nc1wT_sb[:, :, None, :].to_broadcast((128, 4, NW, P))
            nc.vector.tensor_mul(
                out=tmp1.rearrange("p c (s q) -> p c s q", q=P),
                in0=xv, in1=wv)
            hsumT = tpool.tile([128, 4, NW], fp32)
            nc.vector.reduce_sum(
                out=hsumT, in_=tmp1.rearrange("p c (s q) -> p c s q", q=P),
                axis=mybir.AxisListType.X)

            # ---- M1^T chunks + gelu -> h1cT [112, 7, 45] ----
            h1cT = tpool.tile([112, 7, NW], fp32)
            for hc in range(7):
                pm = psB.tile([112, NW], fp32, tag="pm")
                for fc in range(4):
                    lhsT = p1w_sb[:, fc, hc * 112:(hc + 1) * 112]
                    nc.tensor.matmul(pm, lhsT, hsumT[:, fc, :],
                                     start=(fc == 0), stop=(fc == 3))
                nc.scalar.activation(out=h1cT[:, hc, :], in_=pm, func=Gelu,
                                     bias=b1_sb[:, hc:hc + 1], scale=360.0)

            # ---- node conv 2 (64x batch factor folded into nc2w host-side) ----
            tmp2 = tpool.tile([112, 7, NW], fp32)
            h1v = h1cT.rearrange("p c (s q) -> p c s q", q=P)
            w2v = nc2w_sb[:, :, None, :].to_broadcast((112, 7, S2, P))
            nc.vector.tensor_mul(
                out=tmp2.rearrange("p c (s q) -> p c s q", q=P),
                in0=h1v, in1=w2v)
            hs2T = tpool.tile([112, 7, S2], fp32)
            nc.vector.reduce_sum(
                out=hs2T, in_=tmp2.rearrange("p c (s q) -> p c s q", q=P),
                axis=mybir.AxisListType.X)

            # ---- M2^T [28, 15] + gelu ----
            pm2 = psB.tile([H2, S2], fp32, tag="pm")
            for c in range(7):
                nc.tensor.matmul(pm2, p2w_sb[:, c, :], hs2T[:, c, :],
                                 start=(c == 0), stop=(c == 6))
            out2T = tpool.tile([H2, S2], fp32)
            nc.scalar.activation(out=out2T, in_=pm2, func=Gelu,
                                 bias=b2_sb[:, 0:1], scale=120.0)

            # ---- classifier ----
            o2v = out2T.rearrange("h (r q) -> h r q", q=P)
            pc1 = psB.tile([32, CR], fp32, tag="pm")
            for q in range(P):
                nc.tensor.matmul(pc1, cw1_sb[:, q, :], o2v[:, :, q],
                                 start=(q == 0), stop=(q == P - 1))
            c1T = tpool.tile([32, CR], fp32)
            nc.scalar.activation(out=c1T, in_=pc1, func=Gelu,
                                 bias=cb1_sb[:, 0:1], scale=1.0)
            pc2 = psB.tile([NCLS, CR], fp32, tag="pm")
            nc.tensor.matmul(pc2, cw2_sb, c1T, start=True, stop=True)
            outT = tpool.tile([NCLS, CR], fp32)
            nc.scalar.activation(out=outT, in_=pc2, func=Ident,
                                 bias=cb2_sb[:, 0:1], scale=1.0)
            nc.sync.dma_start(out=out.ap(), in_=outT)

    nc.compile()
    return nc


def _prep_in_maps(inputs):
    x = np.ascontiguousarray(np.asarray(inputs["x"], dtype=np.float32))
    nc1_w = np.asarray(inputs["nc1_w"], dtype=np.float32)
    prop1_W = np.asarray(inputs["prop1_W"], dtype=np.float32)
    prop1_b = np.asarray(inputs["prop1_b"], dtype=np.float32)
    nc2_w = np.asarray(inputs["nc2_w"], dtype=np.float32)
    prop2_W = np.asarray(inputs["prop2_W"], dtype=np.float32)
    prop2_b = np.asarray(inputs["prop2_b"], dtype=np.float32)
    cls_w1 = np.asarray(inputs["cls_w1"], dtype=np.float32)
    cls_b1 = np.asarray(inputs["cls_b1"], dtype=np.float32)
    cls_w2 = np.asarray(inputs["cls_w2"], dtype=np.float32)
    cls_b2 = np.asarray(inputs["cls_b2"], dtype=np.float32)

    common = {
        "nc1wT": np.ascontiguousarray(
            nc1_w.T.reshape(4, 128, P).swapaxes(0, 1)),
        "eye128": np.eye(128, dtype=np.float32),
        "p1wr": np.ascontiguousarray(
            prop1_W.reshape(4, 128, H1).swapaxes(0, 1)),
        "b1r": np.ascontiguousarray(prop1_b.reshape(7, 112).T),
        "nc2wr": np.ascontiguousarray(
            (64.0 * nc2_w).astype(np.float32).T.reshape(7, 112, P)
            .swapaxes(0, 1)),
        "p2wr": np.ascontiguousarray(prop2_W.reshape(7, 112, H2)
                                     .swapaxes(0, 1)),
        "b2c": np.ascontiguousarray(prop2_b.reshape(H2, 1)),
        "cw1": np.ascontiguousarray(cls_w1.reshape(P, H2, 32).swapaxes(0, 1)),
        "cb1c": np.ascontiguousarray(cls_b1.reshape(32, 1)),
        "cw2": np.ascontiguousarray(cls_w2),
        "cb2c": np.ascontiguousarray(cls_b2.reshape(NCLS, 1)),
    }
    in_maps = []
    for c in range(NCORES):
        xsl = np.ascontiguousarray(
            x[:, c * SLICE_N:(c + 1) * SLICE_N, :].reshape(B, SLICE_ELEMS))
        in_maps.append({"xs": xsl, **common})
    return in_maps


def run(inputs, trace=False):
    from concourse import bass_utils
    if "nc" not in _CACHE:
        _CACHE["nc"] = _build_bass()
    nc = _CACHE["nc"]
    in_maps = _prep_in_maps(inputs)
    res = bass_utils.run_bass_kernel_spmd(
        nc, in_maps, core_ids=list(range(NCORES)), trace=trace)
    outs = [np.asarray(res.results[c]["out"]) for c in range(NCORES)]
    block = np.concatenate([o.T for o in outs], axis=0)       # [40, 10]
    full = np.tile(block, (B, 1)).astype(np.float32)          # [2560, 10]
    return full, res


def kernel(**inputs) -> np.ndarray:
    out, _ = run(inputs, trace=False)
    return out


# revision 14
# speedup vs baseline: 1.0857x; 1.0300x over previous
"""Trainium2 Bass kernel for nn_DCGN_5239860101881.

Math background (verified against the reference numerically):
  - The DCGN's "adjacency" matrix is diagonal with diag == 1.0 in fp32
    (cos(v,v) path), so einsum('xyz,abc->xbc') makes every propagate output
      out[b] = S * (sum_batch(node_conv(x)) @ W) + bias      (S = 360 / 120)
    and the reference output consists of 64 bit-identical [40,10] blocks.
  - The only computation touching the big x tensor is x.sum(axis=0).

Distribution: shard the node axis (1080 = 8 * 135) across the 8 cores.
Each core streams its [64, 135, 512] slice from HBM (DMA-bound), reduces
over batch, then runs the tiny replicated chain:
  node_conv -> @prop1_W -> gelu(360*. + b1) -> node_conv2(64x folded into w)
  -> @prop2_W -> gelu(120*. + b2) -> classifier
producing 5 of the 40 distinct output rows. No collectives needed.

Implementation notes:
  - Stream tiles are node-major [128 nodes, 512 f] per batch; the batch
    reduction runs on the otherwise-idle TensorE as accumulating PE
    transposes (psum += tile_chunk^T), which also leaves X^T in the
    f-on-partitions layout the tail matmuls need and keeps the PE HAM-warm.
  - Feature chunk 3 and the 7 leftover nodes (128..134) are reduced on DVE;
    leftovers stream in early and their reshuffle/transposes hide under
    phase A entirely.
"""

import numpy as np

B, N, F = 64, 1080, 512
H1, H2, NCLS = 784, 28, 10
P = 3
NCORES = 8
SLICE_N = N // NCORES            # 135 nodes per core
NW = SLICE_N // P                # 45 layer-1 windows per core
S2 = NW // P                     # 15 layer-2 windows per core
CR = S2 // P                     # 5 classifier rows per core
SLICE_ELEMS = SLICE_N * F        # 69120
MAIN_ELEMS = 128 * F             # 65536 (nodes 0..127)
LEFT_ELEMS = SLICE_ELEMS - MAIN_ELEMS  # 3584 (nodes 128..134)
GB = 8                           # batches per DMA group
NGROUPS = B // GB

_CACHE = {}


def _build_bass():
    import concourse.mybir as mybir
    from concourse import bacc
    from concourse.tile import TileContext

    fp32 = mybir.dt.float32
    nc = bacc.Bacc("TRN2", target_bir_lowering=False, debug=False,
                   num_devices=NCORES)

    xs = nc.dram_tensor("xs", [B, SLICE_ELEMS], fp32, kind="ExternalInput")
    nc1wT = nc.dram_tensor("nc1wT", [128, 4, P], fp32, kind="ExternalInput")
    eye128 = nc.dram_tensor("eye128", [128, 128], fp32, kind="ExternalInput")
    p1wr = nc.dram_tensor("p1wr", [128, 4, H1], fp32, kind="ExternalInput")
    b1r = nc.dram_tensor("b1r", [112, 7], fp32, kind="ExternalInput")
    nc2wr = nc.dram_tensor("nc2wr", [112, 7, P], fp32, kind="ExternalInput")
    p2wr = nc.dram_tensor("p2wr", [112, 7, H2], fp32, kind="ExternalInput")
    b2c = nc.dram_tensor("b2c", [H2, 1], fp32, kind="ExternalInput")
    cw1 = nc.dram_tensor("cw1", [H2, P, 32], fp32, kind="ExternalInput")
    cb1c = nc.dram_tensor("cb1c", [32, 1], fp32, kind="ExternalInput")
    cw2 = nc.dram_tensor("cw2", [32, NCLS], fp32, kind="ExternalInput")
    cb2c = nc.dram_tensor("cb2c", [NCLS, 1], fp32, kind="ExternalInput")

    out = nc.dram_tensor("out", [NCLS, CR], fp32, kind="ExternalOutput")

    Gelu = mybir.ActivationFunctionType.Gelu
    Ident = mybir.ActivationFunctionType.Identity

    with TileContext(nc) as tc:
        with (
            tc.tile_pool(name="w", bufs=1) as wpool,
            tc.tile_pool(name="stream", bufs=3) as spool,
            tc.tile_pool(name="left", bufs=1) as lpool,
            tc.tile_pool(name="acc", bufs=1) as apool,
            tc.tile_pool(name="tail", bufs=1) as tpool,
            tc.tile_pool(name="psA", bufs=1, space="PSUM") as psA,
            tc.tile_pool(name="psB", bufs=2, space="PSUM") as psB,
            tc.tile_pool(name="dram", bufs=1, space="DRAM") as dpool,
        ):
            # identity needed by the very first transposes
            eye_sb = wpool.tile([128, 128], fp32)
            nc.scalar.dma_start(out=eye_sb, in_=eye128.ap())

            # leftover node stream (nodes 128..134), all batches, early
            llt = lpool.tile([128, B, 28], fp32)
            for q in range(4):
                src = xs.ap()[q * 16:(q + 1) * 16, MAIN_ELEMS:].rearrange(
                    "b (p f) -> p b f", p=128)
                nc.scalar.dma_start(out=llt[:, q * 16:(q + 1) * 16, :], in_=src)

            # main group DMAs issued up-front in program order
            acc3 = apool.tile([128, 128], fp32)
            px = psA.tile([128, 3, 128], fp32)   # one psum bank
            gts = []
            for g in range(NGROUPS):
                gtm = spool.tile([128, GB, F], fp32, tag="grp")
                src = xs.ap()[g * GB:(g + 1) * GB, 0:MAIN_ELEMS].rearrange(
                    "b (n f) -> n b f", n=128)
                nc.sync.dma_start(out=gtm, in_=src)
                gts.append(gtm)
                for b in range(GB):
                    bg = g * GB + b
                    for fc in range(3):
                        nc.tensor.matmul(
                            px[:, fc, :], gtm[:, b, fc * 128:(fc + 1) * 128],
                            eye_sb,
                            start=(bg == 0 and fc == 0),
                            stop=(bg == B - 1 and fc == 2))
                    if bg == 0:
                        nc.vector.tensor_copy(out=acc3, in_=gtm[:, 0, 384:512])
                    else:
                        nc.vector.tensor_add(out=acc3, in0=acc3,
                                             in1=gtm[:, b, 384:512])

            # ---- weights (scheduled around the stream) ----
            nc1wT_sb = wpool.tile([128, 4, P], fp32)
            nc.scalar.dma_start(out=nc1wT_sb, in_=nc1wT.ap())
            p1w_sb = wpool.tile([128, 4, H1], fp32)
            nc.scalar.dma_start(out=p1w_sb, in_=p1wr.ap())
            b1_sb = wpool.tile([112, 7], fp32)
            nc.scalar.dma_start(out=b1_sb, in_=b1r.ap())
            nc2w_sb = wpool.tile([112, 7, P], fp32)
            nc.scalar.dma_start(out=nc2w_sb, in_=nc2wr.ap())
            p2w_sb = wpool.tile([112, 7, H2], fp32)
            nc.scalar.dma_start(out=p2w_sb, in_=p2wr.ap())
            b2_sb = wpool.tile([H2, 1], fp32)
            nc.scalar.dma_start(out=b2_sb, in_=b2c.ap())
            cw1_sb = wpool.tile([H2, P, 32], fp32)
            nc.scalar.dma_start(out=cw1_sb, in_=cw1.ap())
            cb1_sb = wpool.tile([32, 1], fp32)
            nc.scalar.dma_start(out=cb1_sb, in_=cb1c.ap())
            cw2_sb = wpool.tile([32, NCLS], fp32)
            nc.scalar.dma_start(out=cw2_sb, in_=cw2.ap())
            cb2_sb = wpool.tile([NCLS, 1], fp32)
            nc.scalar.dma_start(out=cb2_sb, in_=cb2c.ap())

            # preload the gelu ACT table during phase A
            gdummy = tpool.tile([H2, 1], fp32)
            nc.scalar.activation(out=gdummy, in_=b2_sb, func=Gelu)

            # X^T assembled here: [f-part 128, fc 4, node 135]
            xT_sb = tpool.tile([128, 4, SLICE_N], fp32)

            # leftover reduction: 63 adds of [128, 28] + roundtrip + transposes
            accl = apool.tile([128, 28], fp32)
            for b in range(B):
                if b == 0:
                    nc.vector.tensor_copy(out=accl, in_=llt[:, 0, :])
                else:
                    nc.vector.tensor_add(out=accl, in0=accl, in1=llt[:, b, :])
            scratch = dpool.tile([LEFT_ELEMS], fp32)
            nc.sync.dma_start(
                out=scratch.rearrange("(p f) -> p f", p=128), in_=accl)
            lt7 = lpool.tile([7, F], fp32)
            nc.sync.dma_start(
                out=lt7, in_=scratch.rearrange("(n f) -> n f", n=7))
            for fc in range(4):
                plt = psB.tile([128, 7], fp32, tag="plt")
                nc.tensor.matmul(plt, lt7[:, fc * 128:(fc + 1) * 128],
                                 eye_sb[:7, :7], start=True, stop=True)
                nc.vector.tensor_copy(out=xT_sb[:, fc, 128:135], in_=plt)

            # ---- drains after the stream ----
            for fc in range(3):
                nc.vector.tensor_copy(out=xT_sb[:, fc, 0:128], in_=px[:, fc, :])
            p3 = psB.tile([128, 128], fp32, tag="p3")
            nc.tensor.matmul(p3, acc3, eye_sb, start=True, stop=True)
            nc.vector.tensor_copy(out=xT_sb[:, 3, 0:128], in_=p3)

            # ---- node conv 1 (transposed layout) ----
            tmp1 = tpool.tile([128, 4, SLICE_N], fp32)
            xv = xT_sb.rearrange("p c (s q) -> p c s q", q=P)
            wv = nc1wT_sb[:, :, None, :].to_broadcast((128, 4, NW, P))
            nc.vector.tensor_mul(
                out=tmp1.rearrange("p c (s q) -> p c s q", q=P),
                in0=xv, in1=wv)
            hsumT = tpool.tile([128, 4, NW], fp32)
            nc.vector.reduce_sum(
                out=hsumT, in_=tmp1.rearrange("p c (s q) -> p c s q", q=P),
                axis=mybir.AxisListType.X)

            # ---- M1^T chunks + gelu -> h1cT [112, 7, 45] ----
            h1cT = tpool.tile([112, 7, NW], fp32)
            for hc in range(7):
                pm = psB.tile([112, NW], fp32, tag="pm")
                for fc in range(4):
                    lhsT = p1w_sb[:, fc, hc * 112:(hc + 1) * 112]
                    nc.tensor.matmul(pm, lhsT, hsumT[:, fc, :],
                                     start=(fc == 0), stop=(fc == 3))
                nc.scalar.activation(out=h1cT[:, hc, :], in_=pm, func=Gelu,
                                     bias=b1_sb[:, hc:hc + 1], scale=360.0)

            # ---- node conv 2 (64x batch factor folded into nc2w host-side) ----
            tmp2 = tpool.tile([112, 7, NW], fp32)
            h1v = h1cT.rearrange("p c (s q) -> p c s q", q=P)
            w2v = nc2w_sb[:, :, None, :].to_broadcast((112, 7, S2, P))
            nc.vector.tensor_mul(
                out=tmp2.rearrange("p c (s q) -> p c s q", q=P),
                in0=h1v, in1=w2v)
            hs2T = tpool.tile([112, 7, S2], fp32)
            nc.vector.reduce_sum(
                out=hs2T, in_=tmp2.rearrange("p c (s q) -> p c s q", q=P),
                axis=mybir.AxisListType.X)

            # ---- M2^T [28, 15] + gelu ----
            pm2 = psB.tile([H2, S2], fp32, tag="pm")
            for c in range(7):
                nc.tensor.matmul(pm2, p2w_sb[:, c, :], hs2T[:, c, :],
                                 start=(c == 0), stop=(c == 6))
            out2T = tpool.tile([H2, S2], fp32)
            nc.scalar.activation(out=out2T, in_=pm2, func=Gelu,
                                 bias=b2_sb[:, 0:1], scale=120.0)

            # ---- classifier ----
            o2v = out2T.rearrange("h (r q) -> h r q", q=P)
            pc1 = psB.tile([32, CR], fp32, tag="pm")
            for q in range(P):
                nc.tensor.matmul(pc1, cw1_sb[:, q, :], o2v[:, :, q],
                                 start=(q == 0), stop=(q == P - 1))
            c1T = tpool.tile([32, CR], fp32)
            nc.scalar.activation(out=c1T, in_=pc1, func=Gelu,
                                 bias=cb1_sb[:, 0:1], scale=1.0)
            pc2 = psB.tile([NCLS, CR], fp32, tag="pm")
            nc.tensor.matmul(pc2, cw2_sb, c1T, start=True, stop=True)
            outT = tpool.tile([NCLS, CR], fp32)
            nc.scalar.activation(out=outT, in_=pc2, func=Ident,
                                 bias=cb2_sb[:, 0:1], scale=1.0)
            nc.sync.dma_start(out=out.ap(), in_=outT)

    nc.compile()
    return nc


def _prep_in_maps(inputs):
    x = np.ascontiguousarray(np.asarray(inputs["x"], dtype=np.float32))
    nc1_w = np.asarray(inputs["nc1_w"], dtype=np.float32)
    prop1_W = np.asarray(inputs["prop1_W"], dtype=np.float32)
    prop1_b = np.asarray(inputs["prop1_b"], dtype=np.float32)
    nc2_w = np.asarray(inputs["nc2_w"], dtype=np.float32)
    prop2_W = np.asarray(inputs["prop2_W"], dtype=np.float32)
    prop2_b = np.asarray(inputs["prop2_b"], dtype=np.float32)
    cls_w1 = np.asarray(inputs["cls_w1"], dtype=np.float32)
    cls_b1 = np.asarray(inputs["cls_b1"], dtype=np.float32)
    cls_w2 = np.asarray(inputs["cls_w2"], dtype=np.float32)
    cls_b2 = np.asarray(inputs["cls_b2"], dtype=np.float32)

    common = {
        "nc1wT": np.ascontiguousarray(
            nc1_w.T.reshape(4, 128, P).swapaxes(0, 1)),
        "eye128": np.eye(128, dtype=np.float32),
        "p1wr": np.ascontiguousarray(
            prop1_W.reshape(4, 128, H1).swapaxes(0, 1)),
        "b1r": np.ascontiguousarray(prop1_b.reshape(7, 112).T),
        "nc2wr": np.ascontiguousarray(
            (64.0 * nc2_w).astype(np.float32).T.reshape(7, 112, P)
            .swapaxes(0, 1)),
        "p2wr": np.ascontiguousarray(prop2_W.reshape(7, 112, H2)
                                     .swapaxes(0, 1)),
        "b2c": np.ascontiguousarray(prop2_b.reshape(H2, 1)),
        "cw1": np.ascontiguousarray(cls_w1.reshape(P, H2, 32).swapaxes(0, 1)),
        "cb1c": np.ascontiguousarray(cls_b1.reshape(32, 1)),
        "cw2": np.ascontiguousarray(cls_w2),
        "cb2c": np.ascontiguousarray(cls_b2.reshape(NCLS, 1)),
    }
    in_maps = []
    for c in range(NCORES):
        xsl = np.ascontiguousarray(
            x[:, c * SLICE_N:(c + 1) * SLICE_N, :].reshape(B, SLICE_ELEMS))
        in_maps.append({"xs": xsl, **common})
    return in_maps


def run(inputs, trace=False):
    from concourse import bass_utils
    if "nc" not in _CACHE:
        _CACHE["nc"] = _build_bass()
    nc = _CACHE["nc"]
    in_maps = _prep_in_maps(inputs)
    res = bass_utils.run_bass_kernel_spmd(
        nc, in_maps, core_ids=list(range(NCORES)), trace=trace)
    outs = [np.asarray(res.results[c]["out"]) for c in range(NCORES)]
    block = np.concatenate([o.T for o in outs], axis=0)       # [40, 10]
    full = np.tile(block, (B, 1)).astype(np.float32)          # [2560, 10]
    return full, res


def kernel(**inputs) -> np.ndarray:
    out, _ = run(inputs, trace=False)
    return out
